# revision 11
# baseline (speedup 1.0000x reference)
"""Trainium2 Bass kernel for CustomMultiHeadAttention (single-query pooled attention).

Reference computation (B=32, S=1024, D=256, H=8):
    keys   = (x @ Wk + bk).reshape(B,S,H,D)
    values = (x @ Wv + bv).reshape(B,S,H,D)
    scores = einsum('bshd,hd->bsh', keys, query)
    attn   = softmax(scores, axis=1)           # over S
    pooled = einsum('bsh,bshd->bhd', attn, values).reshape(B, H*D)
    out    = pooled @ Wo + bo

Algebraic restructure (exact in real arithmetic):
    q_proj[e,h] = sum_d Wk[e, h*D+d] * query[h,d]        # [256, 8]
    scores[b,s,h] = x[b,s,:] @ q_proj[:,h]  (+ const(h) from bk -> cancels in softmax)
    attnu = exp(scores - 64)                             # const shift; softmax invariant
    ctx[b,h,e]  = sum_s attnu[b,s,h] * x[b,s,e];  Z[b,h] = sum_s attnu[b,s,h]
    pooled[b,h,:] = (ctx[b,h,:]/Z[b,h]) @ Wv_h + bv_h    # sum_s attn = 1
    out = pooled @ Wo + (bv @ Wo + bo)

This removes both [B*S,256]x[256,2048] projections; the kernel is memory-bound.
Z is obtained free as an extra all-ones column appended to x in the ctx matmul.
Scores use exact fp32 matmuls (cheap: N=8); the post-softmax path uses float32r.
Sharding: data-parallel over batch, 4 batches per core on 8 cores.

Layout note: PE matmul operands/outputs need base partition in {0,32,64}, so
local batches 0..2 sit at partition offsets 0/32/64 and batch 3 uses a second
free-dim slab at offset 0 (only relevant for the tiny [8 x *] ctx tiles).
"""

import sys

sys.path.insert(0, "/opt/trn_rl_repo")

import numpy as np

import concourse.bass as bass
import concourse.mybir as mybir
import concourse.tile as tile
from concourse import bacc
from concourse.bass_utils import run_bass_kernel_spmd
from concourse.masks import make_identity

F32 = mybir.dt.float32
F32R = mybir.dt.float32r

B, S, D, H = 32, 1024, 256, 8
NCORES = 8
BL = B // NCORES      # local batches per core = 4
ST = S // 128         # s-tiles per batch = 8
KD = 2                # 256 = 2 k-tiles of 128 over the D (input dim) axis
KHD = (H * D) // 128  # 16 k-tiles over the H*D axis
SHIFT = 64.0          # constant score shift before exp (softmax-invariant)

def build_program():
    nc = bacc.Bacc("TRN2", target_bir_lowering=False, debug=False)

    xn_d = nc.dram_tensor("xn", [BL, S, D + 2], F32R, kind="ExternalInput")
    xt_d = nc.dram_tensor("xt", [BL, D, S], F32, kind="ExternalInput")
    wk_d = nc.dram_tensor("wk", [D, H * D], F32, kind="ExternalInput")
    wv_d = nc.dram_tensor("wv", [D, H * D], F32R, kind="ExternalInput")
    wo_d = nc.dram_tensor("wo", [H * D, D], F32R, kind="ExternalInput")
    q_d = nc.dram_tensor("q", [H, D], F32, kind="ExternalInput")
    bv_d = nc.dram_tensor("bv", [H * D], F32, kind="ExternalInput")
    bo_d = nc.dram_tensor("bo", [D], F32R, kind="ExternalInput")
    on_d = nc.dram_tensor("on", [1, BL], F32R, kind="ExternalInput")
    out_d = nc.dram_tensor("out", [BL, D], F32, kind="ExternalOutput")

    with tile.TileContext(nc) as tc:
        with (
            tc.tile_pool(name="big", bufs=1) as big,
            tc.tile_pool(name="sm", bufs=1) as sm,
            tc.tile_pool(name="ps", bufs=1, space=bass.MemorySpace.PSUM) as ps,
            tc.tile_pool(name="pst", bufs=2, space=bass.MemorySpace.PSUM) as pst,
        ):
            # ---- SBUF allocations -------------------------------------
            xn_sb = big.tile([128, BL, ST, D + 2], F32R)  # x natural + 2 ones cols
            xt_sb = big.tile([128, KD, BL, S], F32)       # x transposed: p=e%128
            wk_sb = big.tile([128, KD, H * D], F32)
            wv_sb = big.tile([128, KD, H * D], F32R)
            wo_sb = big.tile([128, KHD, D], F32R)
            qrep = big.tile([128, H * D], F32)            # query replicated
            tmp = big.tile([128, KD, H * D], F32)         # wk * qrep scratch

            qp = sm.tile([128, KD, H], F32)               # q_proj [e, h]
            attn_sb = sm.tile([128, BL, ST, H], F32R)     # exp(scores-SHIFT) [s, h]
            recip = sm.tile([H, BL, 1], F32)              # 1/Z per (h, b)
            ctx_sb = sm.tile([H, BL, D], F32)             # [h, b, e]
            ctxT_sb = sm.tile([128, KD, BL, H], F32R)     # [e%128, eh, b, h]
            pooledT_sb = sm.tile([128, KHD, BL], F32R)    # [(hd)%128, ktile, b]
            bvn_sb = sm.tile([KHD, 128], F32)             # bv natural [k, p]
            bvT_sb = sm.tile([128, KHD], F32R)
            bo_sb = sm.tile([1, D], F32R)
            bias_sb = sm.tile([1, D], F32R)               # bv @ Wo + bo
            ones_sb = sm.tile([1, BL], F32R)
            ident = sm.tile([16, 16], F32)
            negs = sm.tile([128, 1], F32)                 # -SHIFT bias for exp
            out_sb = sm.tile([BL, D], F32)

            # ---- DMA loads -------------------------------------------
            nc.sync.dma_start(
                wk_sb[:], wk_d[:].rearrange("(k p) f -> p k f", p=128)
            )
            nc.sync.dma_start(
                qrep[:],
                q_d[:].rearrange("h d -> () (h d)").broadcast_to([128, H * D]),
            )
            for b in range(BL):
                nc.sync.dma_start(
                    xt_sb[:, :, b, :],
                    xt_d[b].rearrange("(k p) s -> p k s", p=128),
                )
            for b in range(BL):
                nc.sync.dma_start(
                    xn_sb[:, b, :, :],
                    xn_d[b].rearrange("(t p) e -> p t e", p=128),
                )
            nc.sync.dma_start(
                wv_sb[:], wv_d[:].rearrange("(k p) f -> p k f", p=128)
            )
            nc.sync.dma_start(
                wo_sb[:], wo_d[:].rearrange("(k p) n -> p k n", p=128)
            )
            nc.sync.dma_start(bvn_sb[:], bv_d[:].rearrange("(k p) -> k p", p=128))
            nc.sync.dma_start(bo_sb[:], bo_d[:].rearrange("d -> () d"))
            nc.sync.dma_start(ones_sb[:], on_d[:])

            make_identity(nc, ident[:])
            nc.vector.memset(negs[:], -SHIFT)

            # ---- q_proj[e,h] = sum_d Wk[e, h*D+d] * query[h,d] (DVE) --
            nc.vector.tensor_mul(
                tmp[:],
                wk_sb[:],
                qrep[:].rearrange("p f -> p () f").broadcast_to([128, KD, H * D]),
            )
            nc.vector.reduce_sum(
                qp[:],
                tmp[:].rearrange("p k (h d) -> p k h d", d=D),
                axis=mybir.AxisListType.X,
            )

            # ---- scores[s, h] per (b, s-tile) = xt_tile.T @ q_proj ----
            # out[s, h] = sum_e xt[e, s] * qp[e, h]; exact fp32 (N=8 so cheap)
            scores_ps = ps.tile([128, BL, ST, H], F32, tag="scores")
            for b in range(BL):
                for t in range(ST):
                    for k in range(KD):
                        nc.tensor.matmul(
                            scores_ps[:, b, t, :],
                            xt_sb[:, k, b, t * 128:(t + 1) * 128],
                            qp[:, k, :],
                            start=(k == 0),
                            stop=(k == KD - 1),
                        )
                # exp(scores - SHIFT) -> unnormalized attention weights
                nc.scalar.activation(
                    attn_sb[:, b, :, :],
                    scores_ps[:, b, :, :],
                    mybir.ActivationFunctionType.Exp,
                    bias=negs[:],
                )

            # ---- ctx[h, e] & Z per batch: attnu.T @ [x | 1] (PE) ------
            for b in range(BL):
                ctx_ps = pst.tile([H, 512], F32, tag="ctx")
                for t in range(ST):
                    nc.tensor.matmul(
                        ctx_ps[:, 0:D + 2],
                        attn_sb[:, b, t, :],
                        xn_sb[:, b, t, :],
                        start=(t == 0),
                        stop=(t == ST - 1),
                    )
                # 1/Z from the ones column, then fold into ctx
                nc.vector.reciprocal(recip[:, b, :], ctx_ps[:, D:D + 1])
                nc.vector.tensor_scalar_mul(
                    ctx_sb[:, b, :],
                    ctx_ps[:, 0:D],
                    recip[:, b, :],
                )

            # ---- ctxT[e, (b,h)] via PE transpose ----------------------
            for b in range(BL):
                for eh in range(KD):
                    ctp = pst.tile([128, H], F32, tag="tp")
                    nc.tensor.transpose(
                        ctp[:],
                        ctx_sb[:, b, eh * 128:(eh + 1) * 128],
                        ident[:H, :H],
                    )
                    nc.vector.tensor_copy(ctxT_sb[:, eh, b, :], ctp[:])

            # ---- pooledT[(h d), b] = Wv_h.T @ ctx_h.T (PE, f32r) ------
            pooledT_ps = ps.tile([128, KHD, BL], F32, tag="fin")
            for h in range(H):
                for dh in range(2):
                    for k in range(KD):
                        nc.tensor.matmul(
                            pooledT_ps[:, h * 2 + dh, :],
                            wv_sb[:, k, h * D + dh * 128: h * D + (dh + 1) * 128],
                            ctxT_sb[:, k, :, h],
                            start=(k == 0),
                            stop=(k == KD - 1),
                        )
            nc.vector.tensor_copy(pooledT_sb[:], pooledT_ps[:])

            # ---- bias_total = bv @ Wo + bo (PE) -----------------------
            bvt_ps = pst.tile([128, KHD], F32, tag="tp")
            nc.tensor.transpose(bvt_ps[:], bvn_sb[:], ident[:KHD, :KHD])
            nc.vector.tensor_copy(bvT_sb[:], bvt_ps[:])

            bias_ps = ps.tile([1, D], F32, tag="fin")
            for k in range(KHD):
                nc.tensor.matmul(
                    bias_ps[:],
                    bvT_sb[:, k:k + 1],
                    wo_sb[:, k, :],
                    start=(k == 0),
                    stop=False,
                )
            nc.tensor.matmul(
                bias_ps[:],
                ones_sb[0:1, 0:1],
                bo_sb[:],
                start=False,
                stop=True,
            )
            nc.vector.tensor_copy(bias_sb[:], bias_ps[:])

            # ---- out[b, :] = pooled_flat @ Wo + bias_total (PE, f32r) -
            out_ps = ps.tile([BL, D], F32, tag="fin")
            for k in range(KHD):
                nc.tensor.matmul(
                    out_ps[:],
                    pooledT_sb[:, k, :],
                    wo_sb[:, k, :],
                    start=(k == 0),
                    stop=False,
                )
            nc.tensor.matmul(
                out_ps[:],
                ones_sb[:],
                bias_sb[:],
                start=False,
                stop=True,
            )
            nc.vector.tensor_copy(out_sb[:], out_ps[:])
            nc.sync.dma_start(out_d[:], out_sb[:])

    nc.compile()
    return nc


_NC_CACHE = []


def get_nc():
    if not _NC_CACHE:
        _NC_CACHE.append(build_program())
    return _NC_CACHE[0]


def make_in_maps(x, Wk, bk, Wv, bv, query, Wo, bo):
    x = np.ascontiguousarray(x, dtype=np.float32)
    xt = np.ascontiguousarray(x.transpose(0, 2, 1))
    xn1 = np.concatenate(
        [x, np.ones((x.shape[0], x.shape[1], 2), np.float32)], axis=2
    )
    wk = np.ascontiguousarray(Wk, dtype=np.float32)
    wv = np.ascontiguousarray(Wv, dtype=np.float32)
    wo = np.ascontiguousarray(Wo, dtype=np.float32)
    q = np.ascontiguousarray(query, dtype=np.float32)
    bvv = np.ascontiguousarray(bv, dtype=np.float32)
    bob = np.ascontiguousarray(bo, dtype=np.float32)
    in_maps = []
    for c in range(NCORES):
        sl = slice(c * BL, (c + 1) * BL)
        in_maps.append(
            {
                "xn": xn1[sl],
                "xt": xt[sl],
                "wk": wk,
                "wv": wv,
                "wo": wo,
                "q": q,
                "bv": bvv,
                "bo": bob,
                "on": np.ones((1, BL), np.float32),
            }
        )
    return in_maps


def kernel(x, Wk, bk, Wv, bv, query, Wo, bo):
    nc = get_nc()
    in_maps = make_in_maps(x, Wk, bk, Wv, bv, query, Wo, bo)
    res = run_bass_kernel_spmd(nc, in_maps, core_ids=list(range(NCORES)))
    return np.concatenate([res.results[c]["out"] for c in range(NCORES)], axis=0)
